# revision 54
# baseline (speedup 1.0000x reference)
"""Self-contained Trainium2 kernel for nn_BanzhafModule (conv1 -> self-attention -> conv2).

Data-parallel over 8 NeuronCores: each core processes 4 of the 32 (b*a) batch
elements end-to-end; no collectives. Algebraic fusions cut PE work ~2.5x vs a
direct mapping:
  * scores = (hQ)(hK)^T = h G h^T with G = Q K^T folded on host -> one
    projection (m^T = G^T h^T) instead of two, and scores computed directly
    in T-layout (k on partitions) so no PE transposes of the attention map.
  * conv2(o) = (V W2)^T h^T E^T with U = V @ W2c [512,9] folded on host ->
    the 512-wide attn*V matmul and V projection collapse into a 9-wide one.
  * softmax needs no max pass: scores for this model stay in (-88, +88), so
    exp() is overflow/underflow-safe unshifted; the row sums ride along as
    free ones-columns in the z^T operand -- NINE identical copies on p9o
    rows 32..40, so the per-column normalizer needs no partition broadcast
    (and starts at the 32-aligned partition PSUM engine reads require).
All heavy matmuls run bf16 (fp32 PSUM accumulation); conv zero-padding is
realized by host-side im2col (conv1) and clipped-window DMA writes (conv2).

Schedule notes (why the odd issue order): the PE and the fixed-function
queues are in-order, and semaphore wakeups cost ~1.4us, so (a) dep-free
garbage matmuls warm the HAM clock gate during the weight DMAs, (b) psum
pools are 3-deep for the [128,1024] tiles so a tile's exp() drain is never
2-deep-marginal, (c) z^T/p9h hoist ahead of the scores loop (they only need
h), (d) each element's normalize/scatter chain is issued after the next
element's mt projection so the DVE FIFO never blocks PE-feeding copies, and
(e) final() is deferred one element so its 2 matmuls never head-of-line
block the scores stream.
"""

import numpy as np

E = 4          # batch elements per core
NCORES = 8
IMG = 32       # t = v = 32
L = IMG * IMG  # 1024 tokens
P = 512        # planes

_TAPS = [(dy, dx) for dy in range(3) for dx in range(3)]

_built = {}


def _build_nc():
    import concourse.mybir as mybir
    from concourse import bacc
    from concourse.tile import TileContext

    f32, bf16 = mybir.dt.float32, mybir.dt.bfloat16
    AF = mybir.ActivationFunctionType
    ALU = mybir.AluOpType

    nc = bacc.Bacc("TRN2", target_bir_lowering=False, debug=False, num_devices=NCORES)

    i_xcol = nc.dram_tensor("xcol", [E, 9, L], bf16, kind="ExternalInput")
    i_w1 = nc.dram_tensor("W1c", [9, P], bf16, kind="ExternalInput")
    i_g = nc.dram_tensor("Gm", [128, 4, P], bf16, kind="ExternalInput")
    i_u = nc.dram_tensor("Um", [128, 4, 16], bf16, kind="ExternalInput")
    i_w2 = nc.dram_tensor("W2m", [128, 4, 9], bf16, kind="ExternalInput")
    i_b1 = nc.dram_tensor("b1v", [128, 4], f32, kind="ExternalInput")
    i_b2 = nc.dram_tensor("b2v", [1, 1], f32, kind="ExternalInput")
    o_out = nc.dram_tensor("out", [E, L], f32, kind="ExternalOutput")

    ones_col_d = nc.inline_tensor(np.ones((9, 1), np.float32), name="ones_col")

    with TileContext(nc) as tc:
        with (
            tc.tile_pool(name="wts", bufs=1) as wts,
            tc.tile_pool(name="xp", bufs=2) as xp,
            tc.tile_pool(name="hp", bufs=2) as hp,
            tc.tile_pool(name="mp", bufs=2) as mp,
            tc.tile_pool(name="ep", bufs=2) as ep,
            tc.tile_pool(name="ztp", bufs=2) as ztp,
            tc.tile_pool(name="msc", bufs=2) as msc,
            tc.tile_pool(name="fin", bufs=1) as fin,
            tc.tile_pool(name="pmm", bufs=3, space="PSUM") as pmm,
            tc.tile_pool(name="psm", bufs=2, space="PSUM") as psm,
        ):
            # ---- weights / constants (persistent, all pre-laid-out on host) ----
            prefetch = {}

            def load_x(e):
                xc = xp.tile([9, L], bf16, tag="xc", name=f"xc{e}")
                nc.sync.dma_start(xc[:], i_xcol.ap()[e])
                return xc

            # conv1's deps load first so PE real work can start early ...
            wu = wts.tile([128, 640], bf16)
            nc.vector.memset(wu[:], 1.0)
            prefetch[0] = load_x(0)
            w1b = wts.tile([9, P], bf16)
            nc.sync.dma_start(w1b[:], i_w1.ap())
            b1t = wts.tile([128, 4], f32)
            nc.sync.dma_start(b1t[:], i_b1.ap())
            # ... while dep-free garbage matmuls warm the PE HAM clock gate
            # (idle->busy transition needs ~3.4us of sustained activity).
            # dummy activation pulls the lazy ACT_TABLE_LOAD into the
            # startup shadow instead of delaying the first real relu
            dumact = wts.tile([1, 16], f32)
            nc.scalar.activation(dumact[:], wu[0:1, 0:16], AF.Exp)
            wups = pmm.tile([128, 1024], f32, tag="pmm", name="wups")
            for _ in range(7):
                nc.tensor.matmul(
                    wups[:, 0:512], wu[:, 0:128], wu[:, 128:640],
                    start=True, stop=True,
                )
            gm = wts.tile([128, 4, P], bf16)
            nc.sync.dma_start(gm[:], i_g.ap())
            um = wts.tile([128, 4, 16], bf16)
            nc.sync.dma_start(um[:], i_u.ap())
            w2b = wts.tile([128, 4, 9], bf16)
            nc.sync.dma_start(w2b[:], i_w2.ap())
            b2t = wts.tile([1, 1], f32)
            nc.sync.dma_start(b2t[:], i_b2.ap())
            onesf = wts.tile([9, 1], f32)
            nc.sync.dma_start(onesf[:], ones_col_d.ap())
            oncb = wts.tile([9, 1], bf16)
            nc.vector.tensor_copy(oncb[:], onesf[:])

            p9sh = fin.tile([9, E, L], bf16)
            nc.gpsimd.memset(p9sh[:], 0.0)

            def conv1(e):
                xc = prefetch.pop(e)
                if e + 1 < E:
                    prefetch[e + 1] = load_x(e + 1)
                ht = hp.tile([128, 4, L], bf16, tag="H", name=f"ht{e}")
                for ck in range(4):
                    ps = pmm.tile([128, 1024], f32, tag="pmm", name=f"c1p{e}_{ck}")
                    for lg in range(2):
                        nc.tensor.matmul(
                            ps[:, lg * 512:(lg + 1) * 512],
                            w1b[:, ck * 128:(ck + 1) * 128],
                            xc[:, lg * 512:(lg + 1) * 512],
                            start=True, stop=True,
                        )
                    if ck % 2 == 0:
                        nc.scalar.activation(
                            ht[:, ck, :], ps[:], AF.Relu, bias=b1t[:, ck:ck + 1]
                        )
                    else:
                        nc.vector.tensor_scalar(
                            ht[:, ck, :], ps[:], b1t[:, ck:ck + 1], 0.0,
                            ALU.add, ALU.max,
                        )
                return ht

            def mtproj(e, ht):
                mt = mp.tile([128, 4, L], bf16, tag="M", name=f"mt{e}")
                for ec in range(4):
                    ps = pmm.tile([128, 1024], f32, tag="pmm", name=f"mtp{e}_{ec}")
                    for lg in range(2):
                        for dk in range(4):
                            nc.tensor.matmul(
                                ps[:, lg * 512:(lg + 1) * 512],
                                gm[:, dk, ec * 128:(ec + 1) * 128],
                                ht[:, dk, lg * 512:(lg + 1) * 512],
                                start=(dk == 0), stop=(dk == 3),
                            )
                    if ec % 2 == 0:
                        nc.scalar.copy(mt[:, ec, :], ps[:])
                    else:
                        nc.vector.tensor_copy(mt[:, ec, :], ps[:])
                return mt

            def zhpre(e, ht):
                # z^T and p9h only need ht -- hoist them ahead of the scores
                # loop so the post-scores PE block (and kernel tail) shrinks.
                zt = ztp.tile([128, 8, 41], bf16, tag="zT", name=f"zt{e}")
                nc.gpsimd.memset(zt[:], 1.0)  # cols 9..40 stay 1: free row-sums
                # (cols 32..40 -> NINE identical rowsum rows on p9o partitions
                # 32..40, a legal 32-aligned PSUM read -- no broadcast needed)
                zps = psm.tile([128, 8, 10], f32, tag="sm", name=f"zps{e}")
                for kc in range(8):
                    for dk in range(4):
                        nc.tensor.matmul(
                            zps[:, kc, 0:9],
                            ht[:, dk, kc * 128:(kc + 1) * 128],
                            um[:, dk, 0:9],
                            start=(dk == 0), stop=(dk == 3),
                        )
                nc.scalar.copy(zt[:, :, 0:9], zps[:, :, 0:9])
                phf = msc.tile([9, L], f32, tag="phf", name=f"phf{e}")
                for qg in range(2):
                    ph = psm.tile([9, 512], f32, tag="sm", name=f"ph{e}_{qg}")
                    for dk in range(4):
                        nc.tensor.matmul(
                            ph[:], w2b[:, dk, :], ht[:, dk, qg * 512:(qg + 1) * 512],
                            start=(dk == 0), stop=(dk == 3),
                        )
                    nc.scalar.copy(phf[:, qg * 512:(qg + 1) * 512], ph[:])
                return zt, phf

            def scores_po(e, ht, mt, zt, next_fill):
                # scores^T[k, q] = sum_e h[k, e] m[q, e]; exp lands directly in
                # T-layout (k on partitions) -- no PE transposes needed.
                # p9o accumulation MMs ride 2 chunks behind the scores stream
                # (their exp is already drained) so no head-of-line stalls.
                et = ep.tile([128, 8, L], bf16, tag="eT", name=f"et{e}")
                pos = [
                    psm.tile([41, 512], f32, tag="sm", name=f"po{e}_{qg}")
                    for qg in range(2)
                ]

                for kc in range(8):
                    ps = pmm.tile([128, 1024], f32, tag="pmm", name=f"sc{e}_{kc}")
                    for lg in range(2):
                        for dk in range(4):
                            nc.tensor.matmul(
                                ps[:, lg * 512:(lg + 1) * 512],
                                ht[:, dk, kc * 128:(kc + 1) * 128],
                                mt[:, dk, lg * 512:(lg + 1) * 512],
                                start=(dk == 0), stop=(dk == 3),
                            )
                    if kc == 7:
                        # split the last exp so po0's kc7 matmul (needs only
                        # half 0) starts half an activation earlier
                        nc.scalar.activation(
                            et[:, kc, 0:512], ps[:, 0:512], AF.Exp
                        )
                        nc.scalar.activation(
                            et[:, kc, 512:1024], ps[:, 512:1024], AF.Exp
                        )
                    else:
                        nc.scalar.activation(et[:, kc, :], ps[:], AF.Exp)
                for kc in range(8):
                    nc.tensor.matmul(
                        pos[0][:], zt[:, kc, 0:41],
                        et[:, kc, 0:512],
                        start=(kc == 0), stop=(kc == 7),
                    )
                next_fill()  # conv1(e+1) MMs cover the exp(kc7) drain
                for kc in range(8):
                    nc.tensor.matmul(
                        pos[1][:], zt[:, kc, 0:41],
                        et[:, kc, 512:1024],
                        start=(kc == 0), stop=(kc == 7),
                    )
                return et, pos

            def combine(e, phf, pos):
                # row 32 of po = attention row sums; normalize + add p9h.
                rbc9 = msc.tile([9, L], f32, tag="rbc9", name=f"rbc9{e}")
                rs = msc.tile([9, L], f32, tag="rs", name=f"rs{e}")
                p9e = msc.tile([9, L], bf16, tag="p9e", name=f"p9e{e}")
                # po rows 32..40 are nine identical rowsum rows; stage them
                # on ACT (idle here) and reciprocal all 9 rows at once on DVE
                # ([9,512] costs the same as [1,512] -- free-size bound)
                with tc.high_priority():
                    for qg in range(2):
                        sl = slice(qg * 512, (qg + 1) * 512)
                        nc.scalar.copy(rs[0:9, sl], pos[qg][32:41, :])
                        nc.vector.reciprocal_approx_fast(
                            rbc9[0:9, sl], rs[0:9, sl]
                        )
                # scatter each tap row into its shifted, clipped window --
                # split by source image-row half so the first 9 DMAs can fire
                # while the second half is still normalizing; spread across
                # the gpsimd/sync/scalar DMA queues
                def scatter_half(s):
                    for j, (dy, dx) in enumerate(_TAPS):
                        r0, r1 = max(0, 1 - dy), min(IMG, IMG + 1 - dy)
                        c0, c1 = max(0, 1 - dx), min(IMG, IMG + 1 - dx)
                        sa = max(r0 + dy - 1, 16 * s)
                        sb = min(r1 + dy - 1, 16 * (s + 1))
                        if sb <= sa:
                            continue
                        srcw = p9e[j:j + 1, :].rearrange(
                            "o (r w) -> o r w", w=IMG
                        )[:, sa:sb, c0 + dx - 1:c1 + dx - 1]
                        dstw = p9sh[j:j + 1, e, :].rearrange(
                            "o (r w) -> o r w", w=IMG
                        )[:, sa - dy + 1:sb - dy + 1, c0:c1]
                        eng = (nc.gpsimd, nc.sync, nc.scalar)[j % 3]
                        eng.dma_start(dstw, srcw)

                for qg in range(2):
                    sl = slice(qg * 512, (qg + 1) * 512)
                    tmp = msc.tile([9, 512], f32, tag="tmp", name=f"tmp{e}_{qg}")
                    nc.vector.tensor_tensor(
                        tmp[:], pos[qg][0:9, :], rbc9[:, sl], ALU.mult
                    )
                    nc.vector.tensor_tensor(
                        p9e[:, sl], tmp[:], phf[:, sl], ALU.add
                    )
                    scatter_half(qg)

            def final(e):
                acc1 = msc.tile([1, L], f32, tag="acc1", name=f"acc1{e}")
                for lg in range(2):
                    sl = slice(lg * 512, (lg + 1) * 512)
                    psf = psm.tile([1, 512], f32, tag="sm", name=f"psf{e}_{lg}")
                    nc.tensor.matmul(
                        psf[:], oncb[0:9, 0:1], p9sh[0:9, e, sl],
                        start=True, stop=True,
                    )
                    nc.scalar.activation(
                        acc1[0:1, sl], psf[:], AF.Identity, bias=b2t[0:1, 0:1]
                    )
                nc.sync.dma_start(o_out.ap()[e:e + 1, :], acc1[0:1, :])

            ht_c = conv1(0)
            # dep-free filler keeps the PE (and its HAM clock) busy while
            # the first relus drain; nothing else is ready yet
            wups2 = pmm.tile([128, 1024], f32, tag="pmm", name="wups2")
            for _ in range(8):
                nc.tensor.matmul(
                    wups2[:, 0:512], wu[:, 0:128], wu[:, 128:640],
                    start=True, stop=True,
                )
            mt_c = mtproj(0, ht_c)
            for e in range(E):
                if e > 0:
                    final(e - 1)  # deferred so its MMs never head-of-line block
                zt, phf = zhpre(e, ht_c)
                holder = {}

                def next_fill(e=e, holder=holder):
                    if e + 1 < E:
                        holder["ht"] = conv1(e + 1)

                et, pos = scores_po(e, ht_c, mt_c, zt, next_fill)
                if e + 1 < E:
                    mt_n = mtproj(e + 1, holder["ht"])
                combine(e, phf, pos)
                if e + 1 < E:
                    ht_c, mt_c = holder["ht"], mt_n
            final(E - 1)

    nc.compile()
    return nc


def _host_prep(x, W1, b1, Q, K, V, W2, b2):
    import ml_dtypes
    bf = ml_dtypes.bfloat16
    B = x.shape[0] * x.shape[1]
    xf = np.ascontiguousarray(x, np.float32).reshape(B, IMG, IMG)
    xpad = np.zeros((B, IMG + 2, IMG + 2), np.float32)
    xpad[:, 1:-1, 1:-1] = xf
    xcol = np.empty((B, 9, L), np.float32)
    for j, (dy, dx) in enumerate(_TAPS):
        xcol[:, j] = xpad[:, dy:dy + IMG, dx:dx + IMG].reshape(B, L)
    xcolb = np.ascontiguousarray(xcol.astype(bf))
    w1b = np.ascontiguousarray(np.asarray(W1, np.float32).reshape(P, 9).T).astype(bf)
    w2c = np.asarray(W2, np.float32).reshape(P, 9)
    G = (np.asarray(Q, np.float64) @ np.asarray(K, np.float64).T).astype(np.float32)
    U = (np.asarray(V, np.float64) @ w2c.astype(np.float64)).astype(np.float32)
    gm = np.ascontiguousarray(G.reshape(4, 128, P).transpose(1, 0, 2)).astype(bf)
    upad = np.zeros((P, 16), np.float32)
    upad[:, 0:9] = U
    um = np.ascontiguousarray(upad.reshape(4, 128, 16).transpose(1, 0, 2)).astype(bf)
    w2m = np.ascontiguousarray(w2c.reshape(4, 128, 9).transpose(1, 0, 2)).astype(bf)
    b1v = np.ascontiguousarray(np.asarray(b1, np.float32).reshape(4, 128).T)
    b2v = np.asarray(b2, np.float32).reshape(1, 1)
    return xcolb, w1b, gm, um, w2m, b1v, b2v


def kernel(x, W1, b1, Q, K, V, W2, b2):
    from concourse.bass_utils import run_bass_kernel_spmd

    xcolb, w1b, gm, um, w2m, b1v, b2v = _host_prep(x, W1, b1, Q, K, V, W2, b2)
    if "nc" not in _built:
        _built["nc"] = _build_nc()
    nc = _built["nc"]
    in_maps = []
    for c in range(NCORES):
        in_maps.append({
            "xcol": np.ascontiguousarray(xcolb[E * c:E * (c + 1)]),
            "W1c": w1b, "Gm": gm, "Um": um,
            "W2m": w2m, "b1v": b1v, "b2v": b2v,
        })
    res = run_bass_kernel_spmd(nc, in_maps, core_ids=list(range(NCORES)))
    full = np.concatenate([res.results[c]["out"] for c in range(NCORES)], axis=0)
    return np.ascontiguousarray(
        full.reshape(x.shape[0], x.shape[1], IMG, IMG).astype(np.float32)
    )


# revision 55
# speedup vs baseline: 1.0100x; 1.0100x over previous
"""Self-contained Trainium2 kernel for nn_BanzhafModule (conv1 -> self-attention -> conv2).

Data-parallel over 8 NeuronCores: each core processes 4 of the 32 (b*a) batch
elements end-to-end; no collectives. Algebraic fusions cut PE work ~2.5x vs a
direct mapping:
  * scores = (hQ)(hK)^T = h G h^T with G = Q K^T folded on host -> one
    projection (m^T = G^T h^T) instead of two, and scores computed directly
    in T-layout (k on partitions) so no PE transposes of the attention map.
  * conv2(o) = (V W2)^T h^T E^T with U = V @ W2c [512,9] folded on host ->
    the 512-wide attn*V matmul and V projection collapse into a 9-wide one.
  * softmax needs no max pass: scores for this model stay in (-88, +88), so
    exp() is overflow/underflow-safe unshifted; the row sums ride along as
    free ones-columns in the z^T operand -- NINE identical copies on p9o
    rows 32..40, so the per-column normalizer needs no partition broadcast
    (and starts at the 32-aligned partition PSUM engine reads require).
All heavy matmuls run bf16 (fp32 PSUM accumulation); conv zero-padding is
realized by host-side im2col (conv1) and clipped-window DMA writes (conv2).

Schedule notes (why the odd issue order): the PE and the fixed-function
queues are in-order, and semaphore wakeups cost ~1.4us, so (a) dep-free
garbage matmuls warm the HAM clock gate during the weight DMAs, (b) psum
pools are 3-deep for the [128,1024] tiles so a tile's exp() drain is never
2-deep-marginal, (c) z^T/p9h hoist ahead of the scores loop (they only need
h), (d) each element's normalize/scatter chain is issued after the next
element's mt projection so the DVE FIFO never blocks PE-feeding copies, and
(e) final() is deferred one element so its 2 matmuls never head-of-line
block the scores stream.
"""

import numpy as np

E = 4          # batch elements per core
NCORES = 8
IMG = 32       # t = v = 32
L = IMG * IMG  # 1024 tokens
P = 512        # planes

_TAPS = [(dy, dx) for dy in range(3) for dx in range(3)]

_built = {}


def _build_nc():
    import concourse.mybir as mybir
    from concourse import bacc
    from concourse.tile import TileContext

    f32, bf16 = mybir.dt.float32, mybir.dt.bfloat16
    AF = mybir.ActivationFunctionType
    ALU = mybir.AluOpType

    nc = bacc.Bacc("TRN2", target_bir_lowering=False, debug=False, num_devices=NCORES)

    i_xcol = nc.dram_tensor("xcol", [E, 9, L], bf16, kind="ExternalInput")
    i_w1 = nc.dram_tensor("W1c", [9, P], bf16, kind="ExternalInput")
    i_g = nc.dram_tensor("Gm", [128, 4, P], bf16, kind="ExternalInput")
    i_u = nc.dram_tensor("Um", [128, 4, 16], bf16, kind="ExternalInput")
    i_w2 = nc.dram_tensor("W2m", [128, 4, 9], bf16, kind="ExternalInput")
    i_b1 = nc.dram_tensor("b1v", [128, 4], f32, kind="ExternalInput")
    i_b2 = nc.dram_tensor("b2v", [1, 1], f32, kind="ExternalInput")
    o_out = nc.dram_tensor("out", [E, L], f32, kind="ExternalOutput")

    ones_col_d = nc.inline_tensor(np.ones((9, 1), np.float32), name="ones_col")

    with TileContext(nc) as tc:
        with (
            tc.tile_pool(name="wts", bufs=1) as wts,
            tc.tile_pool(name="xp", bufs=2) as xp,
            tc.tile_pool(name="hp", bufs=2) as hp,
            tc.tile_pool(name="mp", bufs=2) as mp,
            tc.tile_pool(name="ep", bufs=2) as ep,
            tc.tile_pool(name="ztp", bufs=2) as ztp,
            tc.tile_pool(name="msc", bufs=2) as msc,
            tc.tile_pool(name="fin", bufs=1) as fin,
            tc.tile_pool(name="pmm", bufs=3, space="PSUM") as pmm,
            tc.tile_pool(name="psm", bufs=2, space="PSUM") as psm,
        ):
            # ---- weights / constants (persistent, all pre-laid-out on host) ----
            prefetch = {}

            def load_x(e):
                xc = xp.tile([9, L], bf16, tag="xc", name=f"xc{e}")
                nc.sync.dma_start(xc[:], i_xcol.ap()[e])
                return xc

            # conv1's deps load first so PE real work can start early ...
            wu = wts.tile([128, 640], bf16)
            nc.vector.memset(wu[:], 1.0)
            prefetch[0] = load_x(0)
            w1b = wts.tile([9, P], bf16)
            nc.sync.dma_start(w1b[:], i_w1.ap())
            b1t = wts.tile([128, 4], f32)
            nc.sync.dma_start(b1t[:], i_b1.ap())
            # ... while dep-free garbage matmuls warm the PE HAM clock gate
            # (idle->busy transition needs ~3.4us of sustained activity).
            # dummy activation pulls the lazy ACT_TABLE_LOAD into the
            # startup shadow instead of delaying the first real relu
            dumact = wts.tile([1, 16], f32)
            nc.scalar.activation(dumact[:], wu[0:1, 0:16], AF.Exp)
            wups = pmm.tile([128, 1024], f32, tag="pmm", name="wups")
            for _ in range(7):
                nc.tensor.matmul(
                    wups[:, 0:512], wu[:, 0:128], wu[:, 128:640],
                    start=True, stop=True,
                )
            gm = wts.tile([128, 4, P], bf16)
            nc.sync.dma_start(gm[:], i_g.ap())
            um = wts.tile([128, 4, 16], bf16)
            nc.sync.dma_start(um[:], i_u.ap())
            w2b = wts.tile([128, 4, 9], bf16)
            nc.sync.dma_start(w2b[:], i_w2.ap())
            b2t = wts.tile([1, 1], f32)
            nc.sync.dma_start(b2t[:], i_b2.ap())
            onesf = wts.tile([9, 1], f32)
            nc.sync.dma_start(onesf[:], ones_col_d.ap())
            oncb = wts.tile([9, 1], bf16)
            nc.vector.tensor_copy(oncb[:], onesf[:])

            p9sh = fin.tile([9, E, L], bf16)
            nc.gpsimd.memset(p9sh[:], 0.0)

            def conv1(e):
                xc = prefetch.pop(e)
                if e + 1 < E:
                    prefetch[e + 1] = load_x(e + 1)
                ht = hp.tile([128, 4, L], bf16, tag="H", name=f"ht{e}")
                for ck in range(4):
                    ps = pmm.tile([128, 1024], f32, tag="pmm", name=f"c1p{e}_{ck}")
                    for lg in range(2):
                        nc.tensor.matmul(
                            ps[:, lg * 512:(lg + 1) * 512],
                            w1b[:, ck * 128:(ck + 1) * 128],
                            xc[:, lg * 512:(lg + 1) * 512],
                            start=True, stop=True,
                        )
                    if ck % 2 == 0:
                        nc.scalar.activation(
                            ht[:, ck, :], ps[:], AF.Relu, bias=b1t[:, ck:ck + 1]
                        )
                    else:
                        nc.vector.tensor_scalar(
                            ht[:, ck, :], ps[:], b1t[:, ck:ck + 1], 0.0,
                            ALU.add, ALU.max,
                        )
                return ht

            def mtproj(e, ht):
                mt = mp.tile([128, 4, L], bf16, tag="M", name=f"mt{e}")
                for ec in range(4):
                    ps = pmm.tile([128, 1024], f32, tag="pmm", name=f"mtp{e}_{ec}")
                    for lg in range(2):
                        for dk in range(4):
                            nc.tensor.matmul(
                                ps[:, lg * 512:(lg + 1) * 512],
                                gm[:, dk, ec * 128:(ec + 1) * 128],
                                ht[:, dk, lg * 512:(lg + 1) * 512],
                                start=(dk == 0), stop=(dk == 3),
                            )
                    if ec % 2 == 0:
                        nc.scalar.copy(mt[:, ec, :], ps[:])
                    else:
                        nc.vector.tensor_copy(mt[:, ec, :], ps[:])
                return mt

            def zhpre(e, ht):
                # z^T and p9h only need ht -- hoist them ahead of the scores
                # loop so the post-scores PE block (and kernel tail) shrinks.
                zt = ztp.tile([128, 8, 41], bf16, tag="zT", name=f"zt{e}")
                nc.gpsimd.memset(zt[:], 1.0)  # cols 9..40 stay 1: free row-sums
                # (cols 32..40 -> NINE identical rowsum rows on p9o partitions
                # 32..40, a legal 32-aligned PSUM read -- no broadcast needed)
                zps = psm.tile([128, 8, 10], f32, tag="sm", name=f"zps{e}")
                for kc in range(8):
                    for dk in range(4):
                        nc.tensor.matmul(
                            zps[:, kc, 0:9],
                            ht[:, dk, kc * 128:(kc + 1) * 128],
                            um[:, dk, 0:9],
                            start=(dk == 0), stop=(dk == 3),
                        )
                nc.scalar.copy(zt[:, :, 0:9], zps[:, :, 0:9])
                phf = msc.tile([9, L], f32, tag="phf", name=f"phf{e}")
                for qg in range(2):
                    ph = psm.tile([9, 512], f32, tag="sm", name=f"ph{e}_{qg}")
                    for dk in range(4):
                        nc.tensor.matmul(
                            ph[:], w2b[:, dk, :], ht[:, dk, qg * 512:(qg + 1) * 512],
                            start=(dk == 0), stop=(dk == 3),
                        )
                    nc.scalar.copy(phf[:, qg * 512:(qg + 1) * 512], ph[:])
                return zt, phf

            def scores_po(e, ht, mt, zt, next_fill):
                # scores^T[k, q] = sum_e h[k, e] m[q, e]; exp lands directly in
                # T-layout (k on partitions) -- no PE transposes needed.
                # p9o accumulation MMs ride 2 chunks behind the scores stream
                # (their exp is already drained) so no head-of-line stalls.
                et = ep.tile([128, 8, L], bf16, tag="eT", name=f"et{e}")
                pos = [
                    psm.tile([41, 512], f32, tag="sm", name=f"po{e}_{qg}")
                    for qg in range(2)
                ]

                for kc in range(8):
                    ps = pmm.tile([128, 1024], f32, tag="pmm", name=f"sc{e}_{kc}")
                    for lg in range(2):
                        for dk in range(4):
                            nc.tensor.matmul(
                                ps[:, lg * 512:(lg + 1) * 512],
                                ht[:, dk, kc * 128:(kc + 1) * 128],
                                mt[:, dk, lg * 512:(lg + 1) * 512],
                                start=(dk == 0), stop=(dk == 3),
                            )
                    nc.scalar.activation(et[:, kc, :], ps[:], AF.Exp)
                for kc in range(8):
                    nc.tensor.matmul(
                        pos[0][:], zt[:, kc, 0:41],
                        et[:, kc, 0:512],
                        start=(kc == 0), stop=(kc == 7),
                    )
                next_fill()  # conv1(e+1) MMs cover the exp(kc7) drain
                for kc in range(8):
                    nc.tensor.matmul(
                        pos[1][:], zt[:, kc, 0:41],
                        et[:, kc, 512:1024],
                        start=(kc == 0), stop=(kc == 7),
                    )
                return et, pos

            def combine(e, phf, pos):
                # row 32 of po = attention row sums; normalize + add p9h.
                rbc9 = msc.tile([9, L], f32, tag="rbc9", name=f"rbc9{e}")
                rs = msc.tile([9, L], f32, tag="rs", name=f"rs{e}")
                p9e = msc.tile([9, L], bf16, tag="p9e", name=f"p9e{e}")
                # po rows 32..40 are nine identical rowsum rows; stage them
                # on ACT (idle here) and reciprocal all 9 rows at once on DVE
                # ([9,512] costs the same as [1,512] -- free-size bound)
                with tc.high_priority():
                    for qg in range(2):
                        sl = slice(qg * 512, (qg + 1) * 512)
                        nc.scalar.copy(rs[0:9, sl], pos[qg][32:41, :])
                        nc.vector.reciprocal_approx_fast(
                            rbc9[0:9, sl], rs[0:9, sl]
                        )
                # scatter each tap row into its shifted, clipped window --
                # split by source image-row half so the first 9 DMAs can fire
                # while the second half is still normalizing; spread across
                # the gpsimd/sync/scalar DMA queues
                def scatter_half(s):
                    for j, (dy, dx) in enumerate(_TAPS):
                        r0, r1 = max(0, 1 - dy), min(IMG, IMG + 1 - dy)
                        c0, c1 = max(0, 1 - dx), min(IMG, IMG + 1 - dx)
                        sa = max(r0 + dy - 1, 16 * s)
                        sb = min(r1 + dy - 1, 16 * (s + 1))
                        if sb <= sa:
                            continue
                        srcw = p9e[j:j + 1, :].rearrange(
                            "o (r w) -> o r w", w=IMG
                        )[:, sa:sb, c0 + dx - 1:c1 + dx - 1]
                        dstw = p9sh[j:j + 1, e, :].rearrange(
                            "o (r w) -> o r w", w=IMG
                        )[:, sa - dy + 1:sb - dy + 1, c0:c1]
                        eng = (nc.gpsimd, nc.sync, nc.scalar)[j % 3]
                        eng.dma_start(dstw, srcw)

                for qg in range(2):
                    sl = slice(qg * 512, (qg + 1) * 512)
                    tmp = msc.tile([9, 512], f32, tag="tmp", name=f"tmp{e}_{qg}")
                    nc.vector.tensor_tensor(
                        tmp[:], pos[qg][0:9, :], rbc9[:, sl], ALU.mult
                    )
                    nc.vector.tensor_tensor(
                        p9e[:, sl], tmp[:], phf[:, sl], ALU.add
                    )
                    scatter_half(qg)

            def final(e):
                acc1 = msc.tile([1, L], f32, tag="acc1", name=f"acc1{e}")
                for lg in range(2):
                    sl = slice(lg * 512, (lg + 1) * 512)
                    psf = psm.tile([1, 512], f32, tag="sm", name=f"psf{e}_{lg}")
                    nc.tensor.matmul(
                        psf[:], oncb[0:9, 0:1], p9sh[0:9, e, sl],
                        start=True, stop=True,
                    )
                    nc.scalar.activation(
                        acc1[0:1, sl], psf[:], AF.Identity, bias=b2t[0:1, 0:1]
                    )
                nc.sync.dma_start(o_out.ap()[e:e + 1, :], acc1[0:1, :])

            ht_c = conv1(0)
            # dep-free filler keeps the PE (and its HAM clock) busy while
            # the first relus drain; nothing else is ready yet
            wups2 = pmm.tile([128, 1024], f32, tag="pmm", name="wups2")
            for _ in range(8):
                nc.tensor.matmul(
                    wups2[:, 0:512], wu[:, 0:128], wu[:, 128:640],
                    start=True, stop=True,
                )
            mt_c = mtproj(0, ht_c)
            for e in range(E):
                if e > 0:
                    final(e - 1)  # deferred so its MMs never head-of-line block
                zt, phf = zhpre(e, ht_c)
                holder = {}

                def next_fill(e=e, holder=holder):
                    if e + 1 < E:
                        holder["ht"] = conv1(e + 1)

                et, pos = scores_po(e, ht_c, mt_c, zt, next_fill)
                if e + 1 < E:
                    mt_n = mtproj(e + 1, holder["ht"])
                combine(e, phf, pos)
                if e + 1 < E:
                    ht_c, mt_c = holder["ht"], mt_n
            final(E - 1)

    nc.compile()
    return nc


def _host_prep(x, W1, b1, Q, K, V, W2, b2):
    import ml_dtypes
    bf = ml_dtypes.bfloat16
    B = x.shape[0] * x.shape[1]
    xf = np.ascontiguousarray(x, np.float32).reshape(B, IMG, IMG)
    xpad = np.zeros((B, IMG + 2, IMG + 2), np.float32)
    xpad[:, 1:-1, 1:-1] = xf
    xcol = np.empty((B, 9, L), np.float32)
    for j, (dy, dx) in enumerate(_TAPS):
        xcol[:, j] = xpad[:, dy:dy + IMG, dx:dx + IMG].reshape(B, L)
    xcolb = np.ascontiguousarray(xcol.astype(bf))
    w1b = np.ascontiguousarray(np.asarray(W1, np.float32).reshape(P, 9).T).astype(bf)
    w2c = np.asarray(W2, np.float32).reshape(P, 9)
    G = (np.asarray(Q, np.float64) @ np.asarray(K, np.float64).T).astype(np.float32)
    U = (np.asarray(V, np.float64) @ w2c.astype(np.float64)).astype(np.float32)
    gm = np.ascontiguousarray(G.reshape(4, 128, P).transpose(1, 0, 2)).astype(bf)
    upad = np.zeros((P, 16), np.float32)
    upad[:, 0:9] = U
    um = np.ascontiguousarray(upad.reshape(4, 128, 16).transpose(1, 0, 2)).astype(bf)
    w2m = np.ascontiguousarray(w2c.reshape(4, 128, 9).transpose(1, 0, 2)).astype(bf)
    b1v = np.ascontiguousarray(np.asarray(b1, np.float32).reshape(4, 128).T)
    b2v = np.asarray(b2, np.float32).reshape(1, 1)
    return xcolb, w1b, gm, um, w2m, b1v, b2v


def kernel(x, W1, b1, Q, K, V, W2, b2):
    from concourse.bass_utils import run_bass_kernel_spmd

    xcolb, w1b, gm, um, w2m, b1v, b2v = _host_prep(x, W1, b1, Q, K, V, W2, b2)
    if "nc" not in _built:
        _built["nc"] = _build_nc()
    nc = _built["nc"]
    in_maps = []
    for c in range(NCORES):
        in_maps.append({
            "xcol": np.ascontiguousarray(xcolb[E * c:E * (c + 1)]),
            "W1c": w1b, "Gm": gm, "Um": um,
            "W2m": w2m, "b1v": b1v, "b2v": b2v,
        })
    res = run_bass_kernel_spmd(nc, in_maps, core_ids=list(range(NCORES)))
    full = np.concatenate([res.results[c]["out"] for c in range(NCORES)], axis=0)
    return np.ascontiguousarray(
        full.reshape(x.shape[0], x.shape[1], IMG, IMG).astype(np.float32)
    )


# revision 56
# speedup vs baseline: 1.0276x; 1.0174x over previous
"""Self-contained Trainium2 kernel for nn_BanzhafModule (conv1 -> self-attention -> conv2).

Data-parallel over 8 NeuronCores: each core processes 4 of the 32 (b*a) batch
elements end-to-end; no collectives. Algebraic fusions cut PE work ~2.5x vs a
direct mapping:
  * scores = (hQ)(hK)^T = h G h^T with G = Q K^T folded on host -> one
    projection (m^T = G^T h^T) instead of two, and scores computed directly
    in T-layout (k on partitions) so no PE transposes of the attention map.
  * conv2(o) = (V W2)^T h^T E^T with U = V @ W2c [512,9] folded on host ->
    the 512-wide attn*V matmul and V projection collapse into a 9-wide one.
  * softmax needs no max pass: scores for this model stay in (-88, +88), so
    exp() is overflow/underflow-safe unshifted; the row sums ride along as
    free ones-columns in the z^T operand -- NINE identical copies on p9o
    rows 32..40, so the per-column normalizer needs no partition broadcast
    (and starts at the 32-aligned partition PSUM engine reads require).
All heavy matmuls run bf16 (fp32 PSUM accumulation); conv zero-padding is
realized by host-side im2col (conv1) and clipped-window DMA writes (conv2).

Schedule notes (why the odd issue order): the PE and the fixed-function
queues are in-order, and semaphore wakeups cost ~1.4us, so (a) dep-free
garbage matmuls warm the HAM clock gate during the weight DMAs, (b) psum
pools are 3-deep for the [128,1024] tiles so a tile's exp() drain is never
2-deep-marginal, (c) z^T/p9h hoist ahead of the scores loop (they only need
h), (d) each element's normalize/scatter chain is issued after the next
element's mt projection so the DVE FIFO never blocks PE-feeding copies, and
(e) final() is deferred one element so its 2 matmuls never head-of-line
block the scores stream.
"""

import numpy as np

E = 4          # batch elements per core
NCORES = 8
IMG = 32       # t = v = 32
L = IMG * IMG  # 1024 tokens
P = 512        # planes

_TAPS = [(dy, dx) for dy in range(3) for dx in range(3)]

_built = {}


def _build_nc():
    import concourse.mybir as mybir
    from concourse import bacc
    from concourse.tile import TileContext

    f32, bf16 = mybir.dt.float32, mybir.dt.bfloat16
    AF = mybir.ActivationFunctionType
    ALU = mybir.AluOpType

    nc = bacc.Bacc("TRN2", target_bir_lowering=False, debug=False, num_devices=NCORES)

    i_xcol = nc.dram_tensor("xcol", [E, 9, L], bf16, kind="ExternalInput")
    i_w1 = nc.dram_tensor("W1c", [9, P], bf16, kind="ExternalInput")
    i_g = nc.dram_tensor("Gm", [128, 4, P], bf16, kind="ExternalInput")
    i_u = nc.dram_tensor("Um", [128, 4, 16], bf16, kind="ExternalInput")
    i_w2 = nc.dram_tensor("W2m", [128, 4, 9], bf16, kind="ExternalInput")
    i_b1 = nc.dram_tensor("b1v", [128, 4], f32, kind="ExternalInput")
    i_b2 = nc.dram_tensor("b2v", [1, 1], f32, kind="ExternalInput")
    o_out = nc.dram_tensor("out", [E, L], f32, kind="ExternalOutput")

    ones_col_d = nc.inline_tensor(np.ones((9, 1), np.float32), name="ones_col")

    with TileContext(nc) as tc:
        with (
            tc.tile_pool(name="wts", bufs=1) as wts,
            tc.tile_pool(name="xp", bufs=2) as xp,
            tc.tile_pool(name="hp", bufs=2) as hp,
            tc.tile_pool(name="mp", bufs=2) as mp,
            tc.tile_pool(name="ep", bufs=2) as ep,
            tc.tile_pool(name="ztp", bufs=2) as ztp,
            tc.tile_pool(name="msc", bufs=2) as msc,
            tc.tile_pool(name="fin", bufs=1) as fin,
            tc.tile_pool(name="pmm", bufs=3, space="PSUM") as pmm,
            tc.tile_pool(name="psm", bufs=2, space="PSUM") as psm,
        ):
            # ---- weights / constants (persistent, all pre-laid-out on host) ----
            prefetch = {}

            def load_x(e):
                xc = xp.tile([9, L], bf16, tag="xc", name=f"xc{e}")
                nc.sync.dma_start(xc[:], i_xcol.ap()[e])
                return xc

            # conv1's deps load first so PE real work can start early ...
            wu = wts.tile([128, 640], bf16)
            nc.vector.memset(wu[:], 1.0)
            prefetch[0] = load_x(0)
            w1b = wts.tile([9, P], bf16)
            nc.sync.dma_start(w1b[:], i_w1.ap())
            b1t = wts.tile([128, 4], f32)
            nc.sync.dma_start(b1t[:], i_b1.ap())
            # ... while dep-free garbage matmuls warm the PE HAM clock gate
            # (idle->busy transition needs ~3.4us of sustained activity).
            # dummy activation pulls the lazy ACT_TABLE_LOAD into the
            # startup shadow instead of delaying the first real relu
            dumact = wts.tile([1, 16], f32)
            nc.scalar.activation(dumact[:], wu[0:1, 0:16], AF.Exp)
            wups = pmm.tile([128, 1024], f32, tag="pmm", name="wups")
            for _ in range(7):
                nc.tensor.matmul(
                    wups[:, 0:512], wu[:, 0:128], wu[:, 128:640],
                    start=True, stop=True,
                )
            gm = wts.tile([128, 4, P], bf16)
            nc.sync.dma_start(gm[:], i_g.ap())
            um = wts.tile([128, 4, 16], bf16)
            nc.sync.dma_start(um[:], i_u.ap())
            w2b = wts.tile([128, 4, 9], bf16)
            nc.sync.dma_start(w2b[:], i_w2.ap())
            b2t = wts.tile([1, 1], f32)
            nc.sync.dma_start(b2t[:], i_b2.ap())
            onesf = wts.tile([9, 1], f32)
            nc.sync.dma_start(onesf[:], ones_col_d.ap())
            oncb = wts.tile([9, 1], bf16)
            nc.vector.tensor_copy(oncb[:], onesf[:])

            p9sh = fin.tile([9, E, L], bf16)
            nc.gpsimd.memset(p9sh[:], 0.0)

            def conv1(e):
                xc = prefetch.pop(e)
                if e + 1 < E:
                    prefetch[e + 1] = load_x(e + 1)
                ht = hp.tile([128, 4, L], bf16, tag="H", name=f"ht{e}")
                for ck in range(4):
                    ps = pmm.tile([128, 1024], f32, tag="pmm", name=f"c1p{e}_{ck}")
                    for lg in range(2):
                        nc.tensor.matmul(
                            ps[:, lg * 512:(lg + 1) * 512],
                            w1b[:, ck * 128:(ck + 1) * 128],
                            xc[:, lg * 512:(lg + 1) * 512],
                            start=True, stop=True,
                        )
                    if ck % 2 == 0:
                        nc.scalar.activation(
                            ht[:, ck, :], ps[:], AF.Relu, bias=b1t[:, ck:ck + 1]
                        )
                    else:
                        nc.vector.tensor_scalar(
                            ht[:, ck, :], ps[:], b1t[:, ck:ck + 1], 0.0,
                            ALU.add, ALU.max,
                        )
                return ht

            def mtproj(e, ht):
                mt = mp.tile([128, 4, L], bf16, tag="M", name=f"mt{e}")
                for ec in range(4):
                    ps = pmm.tile([128, 1024], f32, tag="pmm", name=f"mtp{e}_{ec}")
                    for lg in range(2):
                        for dk in range(4):
                            nc.tensor.matmul(
                                ps[:, lg * 512:(lg + 1) * 512],
                                gm[:, dk, ec * 128:(ec + 1) * 128],
                                ht[:, dk, lg * 512:(lg + 1) * 512],
                                start=(dk == 0), stop=(dk == 3),
                            )
                    if ec % 2 == 0:
                        nc.scalar.copy(mt[:, ec, :], ps[:])
                    else:
                        nc.vector.tensor_copy(mt[:, ec, :], ps[:])
                return mt

            def zhpre(e, ht):
                # z^T and p9h only need ht -- hoist them ahead of the scores
                # loop so the post-scores PE block (and kernel tail) shrinks.
                zt = ztp.tile([128, 8, 41], bf16, tag="zT", name=f"zt{e}")
                nc.gpsimd.memset(zt[:], 1.0)  # cols 9..40 stay 1: free row-sums
                # (cols 32..40 -> NINE identical rowsum rows on p9o partitions
                # 32..40, a legal 32-aligned PSUM read -- no broadcast needed)
                zps = psm.tile([128, 8, 10], f32, tag="sm", name=f"zps{e}")
                for kc in range(8):
                    for dk in range(4):
                        nc.tensor.matmul(
                            zps[:, kc, 0:9],
                            ht[:, dk, kc * 128:(kc + 1) * 128],
                            um[:, dk, 0:9],
                            start=(dk == 0), stop=(dk == 3),
                        )
                nc.scalar.copy(zt[:, :, 0:9], zps[:, :, 0:9])
                phf = msc.tile([9, L], f32, tag="phf", name=f"phf{e}")
                for qg in range(2):
                    ph = psm.tile([9, 512], f32, tag="sm", name=f"ph{e}_{qg}")
                    for dk in range(4):
                        nc.tensor.matmul(
                            ph[:], w2b[:, dk, :], ht[:, dk, qg * 512:(qg + 1) * 512],
                            start=(dk == 0), stop=(dk == 3),
                        )
                    nc.scalar.copy(phf[:, qg * 512:(qg + 1) * 512], ph[:])
                return zt, phf

            def scores_po(e, ht, mt, zt, next_fill):
                # scores^T[k, q] = sum_e h[k, e] m[q, e]; exp lands directly in
                # T-layout (k on partitions) -- no PE transposes needed.
                # p9o accumulation MMs ride 2 chunks behind the scores stream
                # (their exp is already drained) so no head-of-line stalls.
                et = ep.tile([128, 8, L], bf16, tag="eT", name=f"et{e}")
                pos = [
                    psm.tile([41, 512], f32, tag="sm", name=f"po{e}_{qg}")
                    for qg in range(2)
                ]

                for kc in range(8):
                    ps = pmm.tile([128, 1024], f32, tag="pmm", name=f"sc{e}_{kc}")
                    for lg in range(2):
                        for dk in range(4):
                            nc.tensor.matmul(
                                ps[:, lg * 512:(lg + 1) * 512],
                                ht[:, dk, kc * 128:(kc + 1) * 128],
                                mt[:, dk, lg * 512:(lg + 1) * 512],
                                start=(dk == 0), stop=(dk == 3),
                            )
                    nc.scalar.activation(et[:, kc, :], ps[:], AF.Exp)
                for kc in range(8):
                    nc.tensor.matmul(
                        pos[0][:], zt[:, kc, 0:41],
                        et[:, kc, 0:512],
                        start=(kc == 0), stop=(kc == 7),
                    )
                next_fill()  # conv1(e+1) MMs cover the exp(kc7) drain
                for kc in range(8):
                    nc.tensor.matmul(
                        pos[1][:], zt[:, kc, 0:41],
                        et[:, kc, 512:1024],
                        start=(kc == 0), stop=(kc == 7),
                    )
                return et, pos

            def combine(e, phf, pos):
                # row 32 of po = attention row sums; normalize + add p9h.
                rbc9 = msc.tile([9, L], f32, tag="rbc9", name=f"rbc9{e}")
                rs = msc.tile([9, L], f32, tag="rs", name=f"rs{e}")
                p9e = msc.tile([9, L], bf16, tag="p9e", name=f"p9e{e}")
                # po rows 32..40 are nine identical rowsum rows; stage them
                # on ACT (idle here) and reciprocal all 9 rows at once on DVE
                # ([9,512] costs the same as [1,512] -- free-size bound)
                with tc.high_priority():
                    for qg in range(2):
                        sl = slice(qg * 512, (qg + 1) * 512)
                        nc.scalar.copy(rs[0:9, sl], pos[qg][32:41, :])
                        nc.vector.reciprocal_approx_fast(
                            rbc9[0:9, sl], rs[0:9, sl]
                        )
                # scatter each tap row into its shifted, clipped window --
                # split by source image-row half so the first 9 DMAs can fire
                # while the second half is still normalizing; spread across
                # the gpsimd/sync/scalar DMA queues
                def scatter_half(s):
                    for j, (dy, dx) in enumerate(_TAPS):
                        r0, r1 = max(0, 1 - dy), min(IMG, IMG + 1 - dy)
                        c0, c1 = max(0, 1 - dx), min(IMG, IMG + 1 - dx)
                        sa = max(r0 + dy - 1, 16 * s)
                        sb = min(r1 + dy - 1, 16 * (s + 1))
                        if sb <= sa:
                            continue
                        srcw = p9e[j:j + 1, :].rearrange(
                            "o (r w) -> o r w", w=IMG
                        )[:, sa:sb, c0 + dx - 1:c1 + dx - 1]
                        dstw = p9sh[j:j + 1, e, :].rearrange(
                            "o (r w) -> o r w", w=IMG
                        )[:, sa - dy + 1:sb - dy + 1, c0:c1]
                        eng = (nc.gpsimd, nc.sync, nc.scalar)[j % 3]
                        eng.dma_start(dstw, srcw)

                for qg in range(2):
                    sl = slice(qg * 512, (qg + 1) * 512)
                    tmp = msc.tile([9, 512], f32, tag="tmp", name=f"tmp{e}_{qg}")
                    nc.vector.tensor_tensor(
                        tmp[:], pos[qg][0:9, :], rbc9[:, sl], ALU.mult
                    )
                    nc.vector.tensor_tensor(
                        p9e[:, sl], tmp[:], phf[:, sl], ALU.add
                    )
                    scatter_half(qg)

            def final(e):
                acc1 = msc.tile([1, L], f32, tag="acc1", name=f"acc1{e}")
                for lg in range(2):
                    sl = slice(lg * 512, (lg + 1) * 512)
                    psf = psm.tile([1, 512], f32, tag="sm", name=f"psf{e}_{lg}")
                    nc.tensor.matmul(
                        psf[:], oncb[0:9, 0:1], p9sh[0:9, e, sl],
                        start=True, stop=True,
                    )
                    nc.scalar.activation(
                        acc1[0:1, sl], psf[:], AF.Identity, bias=b2t[0:1, 0:1]
                    )
                nc.sync.dma_start(o_out.ap()[e:e + 1, :], acc1[0:1, :])

            ht_c = conv1(0)
            # dep-free filler keeps the PE (and its HAM clock) busy while
            # the first relus drain; lives in the small-psum pool so its
            # slot never waits on a conv1 relu drain (pmm rotation would)
            wups2 = psm.tile([128, 512], f32, tag="sm", name="wups2")
            for _ in range(8):
                nc.tensor.matmul(
                    wups2[:], wu[:, 0:128], wu[:, 128:640],
                    start=True, stop=True,
                )
            mt_c = mtproj(0, ht_c)
            for e in range(E):
                if e > 0:
                    final(e - 1)  # deferred so its MMs never head-of-line block
                zt, phf = zhpre(e, ht_c)
                holder = {}

                def next_fill(e=e, holder=holder):
                    if e + 1 < E:
                        holder["ht"] = conv1(e + 1)

                et, pos = scores_po(e, ht_c, mt_c, zt, next_fill)
                if e + 1 < E:
                    mt_n = mtproj(e + 1, holder["ht"])
                combine(e, phf, pos)
                if e + 1 < E:
                    ht_c, mt_c = holder["ht"], mt_n
            final(E - 1)

    nc.compile()
    return nc


def _host_prep(x, W1, b1, Q, K, V, W2, b2):
    import ml_dtypes
    bf = ml_dtypes.bfloat16
    B = x.shape[0] * x.shape[1]
    xf = np.ascontiguousarray(x, np.float32).reshape(B, IMG, IMG)
    xpad = np.zeros((B, IMG + 2, IMG + 2), np.float32)
    xpad[:, 1:-1, 1:-1] = xf
    xcol = np.empty((B, 9, L), np.float32)
    for j, (dy, dx) in enumerate(_TAPS):
        xcol[:, j] = xpad[:, dy:dy + IMG, dx:dx + IMG].reshape(B, L)
    xcolb = np.ascontiguousarray(xcol.astype(bf))
    w1b = np.ascontiguousarray(np.asarray(W1, np.float32).reshape(P, 9).T).astype(bf)
    w2c = np.asarray(W2, np.float32).reshape(P, 9)
    G = (np.asarray(Q, np.float64) @ np.asarray(K, np.float64).T).astype(np.float32)
    U = (np.asarray(V, np.float64) @ w2c.astype(np.float64)).astype(np.float32)
    gm = np.ascontiguousarray(G.reshape(4, 128, P).transpose(1, 0, 2)).astype(bf)
    upad = np.zeros((P, 16), np.float32)
    upad[:, 0:9] = U
    um = np.ascontiguousarray(upad.reshape(4, 128, 16).transpose(1, 0, 2)).astype(bf)
    w2m = np.ascontiguousarray(w2c.reshape(4, 128, 9).transpose(1, 0, 2)).astype(bf)
    b1v = np.ascontiguousarray(np.asarray(b1, np.float32).reshape(4, 128).T)
    b2v = np.asarray(b2, np.float32).reshape(1, 1)
    return xcolb, w1b, gm, um, w2m, b1v, b2v


def kernel(x, W1, b1, Q, K, V, W2, b2):
    from concourse.bass_utils import run_bass_kernel_spmd

    xcolb, w1b, gm, um, w2m, b1v, b2v = _host_prep(x, W1, b1, Q, K, V, W2, b2)
    if "nc" not in _built:
        _built["nc"] = _build_nc()
    nc = _built["nc"]
    in_maps = []
    for c in range(NCORES):
        in_maps.append({
            "xcol": np.ascontiguousarray(xcolb[E * c:E * (c + 1)]),
            "W1c": w1b, "Gm": gm, "Um": um,
            "W2m": w2m, "b1v": b1v, "b2v": b2v,
        })
    res = run_bass_kernel_spmd(nc, in_maps, core_ids=list(range(NCORES)))
    full = np.concatenate([res.results[c]["out"] for c in range(NCORES)], axis=0)
    return np.ascontiguousarray(
        full.reshape(x.shape[0], x.shape[1], IMG, IMG).astype(np.float32)
    )
